# revision 12
# baseline (speedup 1.0000x reference)
"""CTC forward-loss kernel for 8 Trainium2 NeuronCores (Bass/Tile).

Fixed-lane wavefront (v2): lane s = time-segment index (16 lanes x 8 batch
rows = 128 partitions), cell (u, s) of the label-column x segment grid runs
at wave v = u + K*s. Cross-column feeds (the SW/SQ windows) are same-lane
free-dim slices of the previous wave's output buffer, so no big per-wave
partition-shift DMAs are needed; only the four scan tail values ([128,2] x2)
hop one lane per wave via tiny shifted DMAs, with K-wave slack to hide
latency.

Per wave, per cell: two max-plus scans (VE, W) build a Viterbi normalization
surface; exp-domain scans (PE, Q) propagate the true forward recurrence
normalized by that surface (coefficients A, AO, DL from exp of surface
differences; rho piecewise-constant offsets corrected at segment/column
boundaries via jmp/c2s/c2b).
"""

import os
import sys
import numpy as np
from contextlib import ExitStack

T, B, C, S = 1000, 64, 28, 200
NCORES = 8
BC = B // NCORES
L = 64
NSEG = 16
LANES = 16
K = int(os.environ.get("CTC_K", 2))          # wave skew per segment
NW = (S - 1) + K * (NSEG - 1) + 1
LAG = 2                                       # exp sweep trails V sweep
C1, C2, OFF = 0.28, 1.3, 15.0
NEG = -1.0e30
EMINV = -1.0e9                                # emission for invalid cells
R = 8                                         # Z ring depth (slots)
PR = 4                                        # tail ring depth

SEC_VE, SEC_W, SEC_PE, SEC_Q = 0, 65, 130, 195
ZW = 4 * 65


def _rho(s, u):
    tmid = min(s * L + L // 2, T - 1)
    return np.float32(min(C1 * tmid, C2 * u) + OFF)


def host_prep(prediction, target):
    """Per-core input planes in fixed-lane layout."""
    pred = np.asarray(prediction, dtype=np.float32)
    tgt = np.maximum(np.asarray(target).astype(np.int64) - 1, 0)
    emitE = np.take_along_axis(
        pred, np.broadcast_to(tgt[None], (T, B, S)), axis=2
    ).astype(np.float32)                      # [T,B,S]
    blank = pred[:, :, C - 1].astype(np.float32)   # [T,B]
    TP = NSEG * L
    emitP = np.zeros((TP, B, S), np.float32); emitP[:T] = emitE
    blankP = np.zeros((TP, B), np.float32); blankP[:T] = blank

    emw = np.full((NCORES, 128, NW * L), EMINV, np.float32)
    blk64 = np.zeros((NCORES, 128, L), np.float32)
    jmp = np.zeros((128, NW), np.float32)
    njmp = np.zeros((128, NW), np.float32)
    c2s = np.zeros((128, NW), np.float32)
    c2b = np.full((128, NW), NEG, np.float32)

    seg = np.arange(NSEG) * L
    for s in range(LANES):
        p0, p1 = s * BC, (s + 1) * BC
        for c in range(NCORES):
            g0, g1 = c * BC, (c + 1) * BC
            blk64[c, p0:p1, :] = blankP[seg[s]:seg[s] + L, g0:g1].T
        for v in range(NW):
            u = v - K * s
            if not (0 <= u < S):
                continue
            for c in range(NCORES):
                g0 = c * BC
                emw[c, p0:p1, v * L:(v + 1) * L] = \
                    emitP[seg[s]:seg[s] + L, g0:g0 + BC, u].T
            jmp[p0:p1, v] = _rho(s, u) - (_rho(s - 1, u) if s > 0 else 0.0)
            if s > 0:
                njmp[p0:p1, v] = -jmp[p0, v]
            c2s[p0:p1, v] = (_rho(s, u - 1) - _rho(s, u)) if u >= 1 else 0.0
            if u >= 1 and s >= 1:
                c2b[p0:p1, v] = _rho(s - 1, u - 1) - _rho(s - 1, u)

    ins = []
    for c in range(NCORES):
        ins.append({
            "emw": np.ascontiguousarray(emw[c]),
            "blk64": np.ascontiguousarray(blk64[c]),
            "jmp": jmp, "njmp": njmp, "c2s": c2s, "c2b": c2b,
        })
    return ins


def readout_cells(pl, tl):
    """Per batch row: (g, u_e, s_b, lane, wave, elem)."""
    out = []
    for g in range(B):
        t_b = int(pl[g]) - 1
        u_e = int(tl[g]) - 1
        s_b = t_b // L
        out.append((g, u_e, s_b, s_b, u_e + K * s_b, t_b - s_b * L))
    return out


def build_kernel_body(tc, outs, ins, rcells):
    import concourse.bass as bass
    import concourse.tile as tile
    from concourse import mybir

    nc = tc.nc
    f32 = mybir.dt.float32
    Alu = mybir.AluOpType
    Act = mybir.ActivationFunctionType

    # engine map (env-tunable): v = vector(DVE), g = gpsimd(Pool), s = scalar
    emap = {}
    for key, dflt in (("aarg", "v"), ("aoarg", "v"), ("d", "g"),
                      ("dlarg", "g"), ("dl0", "g"), ("d0", "g"),
                      ("clamp", "g"), ("feed", "g"), ("car", "g"),
                      ("cartv", "v"), ("guard", "g")):
        emap[key] = os.environ.get("CTC_E_" + key.upper(), dflt)
    pick = {"v": nc.vector, "g": nc.gpsimd, "s": nc.scalar}
    E = {k: pick[v] for k, v in emap.items()}

    ctx = ExitStack()
    planes = ctx.enter_context(tc.tile_pool(name="planes", bufs=1))
    rings = ctx.enter_context(tc.tile_pool(name="rings", bufs=1))
    small = ctx.enter_context(tc.tile_pool(name="small", bufs=3))

    emw = planes.tile([128, NW * L], f32)
    blk64 = planes.tile([128, L], f32)
    jmp = planes.tile([128, NW], f32)
    njmp = planes.tile([128, NW], f32)
    c2s = planes.tile([128, NW], f32)
    c2b = planes.tile([128, NW], f32)
    NCH = 16
    chunk = (NW * L + NCH - 1) // NCH
    for i in range(NCH):
        lo, hi = i * chunk, min((i + 1) * chunk, NW * L)
        nc.sync.dma_start(out=emw[:, lo:hi], in_=ins["emw"][:, lo:hi])
    for t_sb, t_dr in ((blk64, ins["blk64"]), (jmp, ins["jmp"]),
                       (njmp, ins["njmp"]), (c2s, ins["c2s"]),
                       (c2b, ins["c2b"])):
        nc.sync.dma_start(out=t_sb, in_=t_dr)

    # Z ring: per wave slot [128, 4*65]: sections VE|W|PE|Q = [guard, d0..d63]
    Z = [rings.tile([128, ZW], f32, name=f"Z{r}") for r in range(R)]
    for r in range(R):
        nc.vector.memset(Z[r], 0.0)
        nc.vector.memset(Z[r][:, SEC_W:SEC_W + 65], NEG)
    # packed shifted-tail ring: cols [VEtail, Wtail, PEtail, Qtail].
    # Lane-0 rows are never DMA'd: statically 0 (V side) / 1 (exp side),
    # so scan initials can read these slots directly (no masking op).
    ps = [rings.tile([128, 4], f32, name=f"ps{r}") for r in range(PR)]
    for r in range(PR):
        nc.vector.memset(ps[r], 0.0)
        nc.vector.memset(ps[r][0:BC, 2:4], 1.0)

    # constants
    negSW = planes.tile([128, L + 1], f32); nc.vector.memset(negSW, NEG)
    zeros64 = planes.tile([128, L + 1], f32); nc.vector.memset(zeros64, 0.0)
    zero1 = planes.tile([128, 1], f32); nc.vector.memset(zero1, 0.0)

    # per-wave work rings (persistent tiles; ring depth managed explicitly)
    Db = [rings.tile([128, L], f32, name=f"D{i}") for i in range(2)]
    SWc = [rings.tile([128, L], f32, name=f"SWc{i}") for i in range(2)]
    ARG = [rings.tile([128, 2 * L], f32, name=f"ARG{i}") for i in range(2)]
    EXPS = [rings.tile([128, 3 * L], f32, name=f"EXPS{i}") for i in range(4)]
    FEEDb = [rings.tile([128, L], f32, name=f"FEED{i}") for i in range(2)]

    rd_by_wave = {}
    for (g, u_e, s_b, lane, wave, elem) in rcells:
        rd_by_wave.setdefault(wave, []).append((g, lane, elem))

    lane_start = {K * s: s for s in range(1, LANES)}   # wave -> lane with u=0

    nwaves = int(os.environ.get("CTC_NWAVES", NW))
    for v in range(min(NW, nwaves) + LAG):
        if v < min(NW, nwaves):
            zv = Z[v % R]
            zp = Z[(v - 1) % R]
            # lane-start isolation: wipe SW window garbage of the invalid
            # predecessor column before this lane's first valid cell reads it
            s0 = lane_start.get(v)
            if s0 is not None:
                nc.sync.dma_start(
                    out=zp[s0 * BC:(s0 + 1) * BC, SEC_W:SEC_W + 65],
                    in_=negSW[s0 * BC:(s0 + 1) * BC, 0:65])
            pcar = ps[(v - K) % PR]
            zsrc = Z[(v - K) % R]
            # W guard of slot v = shifted W tail of wave v-K, straight from
            # that wave's Z slot. Lane0 rows are never written: the NEG
            # init persists there (= no feed), replacing the mask/bias op.
            nc.gpsimd.dma_start(out=zv[BC:128, SEC_W:SEC_W + 1],
                                in_=zsrc[0:128 - BC, SEC_W + 64:SEC_W + 65])
            SW = negSW[:, 0:L] if v == 0 else zp[:, SEC_W:SEC_W + L]
            # V sweep scans (initial = shifted tails; lane0 rows statically 0)
            nc.vector.tensor_tensor_scan(zv[:, SEC_VE + 1:SEC_VE + 65], SW,
                                         emw[:, v * L:(v + 1) * L],
                                         pcar[:, 0:1], Alu.max, Alu.add)
            nc.vector.tensor_tensor_scan(zv[:, SEC_W + 1:SEC_W + 65], blk64,
                                         zv[:, SEC_VE + 1:SEC_VE + 65],
                                         pcar[:, 1:2], Alu.add, Alu.max)
            # tail shift VW: [VEtail, Wtail] cols 64,129 -> lanes +1
            nc.gpsimd.dma_start(out=ps[v % PR][BC:128, 0:2],
                                in_=zv[0:128 - BC, SEC_VE + 64:SEC_W + 65:65])
            # readout of the W surface
            for (g, lane, elem) in rd_by_wave.get(v, ()):
                nc.sync.dma_start(
                    out=outs["outW"][g:g + 1, 0:BC],
                    in_=zv[lane * BC:(lane + 1) * BC,
                           SEC_W + 1 + elem:SEC_W + 2 + elem])
            # coefficient slabs.  D[0] elem via the (raw) guard equals
            # carW - W[0] for lanes>=1; lane0 needs (carW=0) - jmp - W[0],
            # patched by a [8,1] STT before the Aarg/AOarg reads.
            E["d"].tensor_tensor(out=Db[v % 2], in0=zv[:, SEC_W:SEC_W + L],
                                 in1=zv[:, SEC_W + 1:SEC_W + 1 + L],
                                 op=Alu.subtract)
            nc.vector.scalar_tensor_tensor(
                out=Db[v % 2][0:BC, 0:1], in0=zero1[0:BC, :],
                scalar=jmp[0:BC, v:v + 1], in1=zv[0:BC, SEC_W + 1:SEC_W + 2],
                op0=Alu.subtract, op1=Alu.subtract)
            E["aarg"].tensor_tensor(out=ARG[v % 2][:, 0:L],
                                    in0=emw[:, v * L:(v + 1) * L],
                                    in1=Db[v % 2], op=Alu.add)
            E["aoarg"].tensor_tensor(out=ARG[v % 2][:, L:2 * L], in0=blk64,
                                     in1=Db[v % 2], op=Alu.add)
            # SWc = SW - W[t-1]; DL = exp(SWc + c2s) via activation bias
            E["dlarg"].tensor_tensor(out=SWc[v % 2], in0=SW,
                                     in1=zv[:, SEC_W:SEC_W + L],
                                     op=Alu.subtract)
            # A|AO exp, then re-exp elem0 of A and AO with the -jmp
            # boundary bias (njmp is 0 for lane0: its D[0] already exact)
            nc.scalar.activation(out=EXPS[v % 4][:, 0:2 * L],
                                 in_=ARG[v % 2], func=Act.Exp)
            nc.scalar.activation(out=EXPS[v % 4][:, 0:2 * L:L],
                                 in_=ARG[v % 2][:, 0:2 * L:L], func=Act.Exp,
                                 bias=njmp[:, v:v + 1], scale=1.0)
            # DL main (elems 1..63, bias c2s) and DL elem0 (bias c2b)
            nc.scalar.activation(out=EXPS[v % 4][:, 2 * L + 1:3 * L],
                                 in_=SWc[v % 2][:, 1:L], func=Act.Exp,
                                 bias=c2s[:, v:v + 1], scale=1.0)
            nc.scalar.activation(out=EXPS[v % 4][:, 2 * L:2 * L + 1],
                                 in_=SWc[v % 2][:, 0:1], func=Act.Exp,
                                 bias=c2b[:, v:v + 1], scale=1.0)

        # ---------------- exp sweep: wave w2 = v - LAG ----------------
        w2 = v - LAG
        if w2 >= 0:
            zw = Z[w2 % R]
            zwp = Z[(w2 - 1) % R]
            pcarq = ps[(w2 - K) % PR]
            SQ = zeros64[:, 0:L] if w2 == 0 else zwp[:, SEC_Q:SEC_Q + L]
            E["feed"].tensor_tensor(out=FEEDb[w2 % 2],
                                    in0=EXPS[w2 % 4][:, 2 * L:3 * L],
                                    in1=SQ, op=Alu.mult)
            nc.vector.tensor_tensor_scan(zw[:, SEC_PE + 1:SEC_PE + 65],
                                         FEEDb[w2 % 2], EXPS[w2 % 4][:, 0:L],
                                         pcarq[:, 2:3],
                                         Alu.add, Alu.mult)
            nc.vector.tensor_tensor_scan(zw[:, SEC_Q + 1:SEC_Q + 65],
                                         EXPS[w2 % 4][:, L:2 * L],
                                         zw[:, SEC_PE + 1:SEC_PE + 65],
                                         pcarq[:, 3:4],
                                         Alu.mult, Alu.add)
            # lane-start isolation for the SQ window (after the invalid
            # predecessor's Q scan above, before next wave's FEED reads it)
            s0q = lane_start.get(w2 + 1)
            if s0q is not None:
                nc.sync.dma_start(
                    out=zw[s0q * BC:(s0q + 1) * BC, SEC_Q:SEC_Q + 65],
                    in_=zeros64[s0q * BC:(s0q + 1) * BC, 0:65])
            nc.gpsimd.dma_start(out=ps[w2 % PR][BC:128, 2:4],
                                in_=zw[0:128 - BC, SEC_PE + 64:SEC_Q + 65:65])
            # Q guard of slot v (current V wave) = shifted Q tail of
            # exp-wave v-K, read from its Z slot after this iteration's
            # Q scan wrote it. Lane0 keeps the 0 init (dead: DL[0]=0).
            if v < min(NW, nwaves):
                nc.gpsimd.dma_start(
                    out=zv[BC:128, SEC_Q:SEC_Q + 1],
                    in_=Z[(v - K) % R][0:128 - BC, SEC_Q + 64:SEC_Q + 65])
            for (g, lane, elem) in rd_by_wave.get(w2, ()):
                nc.sync.dma_start(
                    out=outs["outP"][g:g + 1, 0:BC],
                    in_=zw[lane * BC:(lane + 1) * BC,
                           SEC_PE + 1 + elem:SEC_PE + 2 + elem])
    ctx.close()


def _build_program(rcells):
    import concourse.bacc as bacc
    import concourse.tile as tile_mod
    from concourse import mybir

    nc = bacc.Bacc("TRN2", target_bir_lowering=False, debug=False,
                   num_devices=NCORES)
    f32 = mybir.dt.float32
    ins = {
        "emw": nc.declare_dram_parameter("emw", [128, NW * L], f32, isOutput=False).ap(),
        "blk64": nc.declare_dram_parameter("blk64", [128, L], f32, isOutput=False).ap(),
        "jmp": nc.declare_dram_parameter("jmp", [128, NW], f32, isOutput=False).ap(),
        "njmp": nc.declare_dram_parameter("njmp", [128, NW], f32, isOutput=False).ap(),
        "c2s": nc.declare_dram_parameter("c2s", [128, NW], f32, isOutput=False).ap(),
        "c2b": nc.declare_dram_parameter("c2b", [128, NW], f32, isOutput=False).ap(),
    }
    outs = {
        "outP": nc.declare_dram_parameter("outP", [B, BC], f32, isOutput=True).ap(),
        "outW": nc.declare_dram_parameter("outW", [B, BC], f32, isOutput=True).ap(),
    }
    with tile_mod.TileContext(nc) as tc:
        build_kernel_body(tc, outs, ins, rcells)
    nc.compile()
    return nc


def kernel(prediction, target, pred_lens, target_lens):
    sys.path.insert(0, "/opt/trn_rl_repo")
    from concourse.bass_utils import run_bass_kernel_spmd

    pl = np.asarray(pred_lens).astype(np.int64)
    tl = np.asarray(target_lens).astype(np.int64)
    rcells = readout_cells(pl, tl)
    in_maps = host_prep(prediction, target)

    nc = _build_program(rcells)
    res = run_bass_kernel_spmd(nc, in_maps, list(range(NCORES)),
                               tmpdir=os.environ.get("CTC_TMPDIR"))
    global LAST_RESULTS
    LAST_RESULTS = res

    total = 0.0
    for (g, u_e, s_b, lane, wave, elem) in rcells:
        c, k = g // BC, g % BC
        pv = np.float64(res.results[c]["outP"][g, k])
        wv = np.float64(res.results[c]["outW"][g, k])
        total += np.log(pv) + wv + float(_rho(s_b, u_e))
    return np.float32(total)
